# revision 7
# baseline (speedup 1.0000x reference)
"""CTC loss kernel for Trainium2 (8 NeuronCores, data-parallel over batch).

Algorithm: the CTC forward DP alpha[t, s] runs as 33 "layer scans" on DVE:
for each extended-target position s, the time recursion
    A[t, s] = (A[t-1, s-1] + m[s] * A[t-1, s-2] + A[t-1, s]) * E[t, s]
is a first-order affine recurrence in t evaluated with one stock DVE
`tensor_tensor_scan` per layer (state = (d0 + state) * d1), both sample
groups concatenated into a single 513-column scan.

v2 changes vs the first working version:
  - feeds shipped in bf16 (half the HBM traffic),
  - the scan feed is shipped transposed ([label-row, time]) so every
    layer's d1 operand is contiguous,
  - the 37-class softmax-denominator reduction runs on the Tensor engine
    as 37 identity-stationary matmuls accumulating into one PSUM bank
    (frees ~21us of DVE time),
  - Ln reads the PSUM bank directly on ACT,
  - scans start after a small 3-row head chunk of the feed is loaded.

Scaled linear domain: E = exp(logit - ln2); the 2^-1 scaling cancels in
ll = log(A_final) - sum_t log(sum37).  Host side: shard batch 2048 ->
8 cores x 256 samples (2 groups of 128 partitions), pre-gather the 17
per-sample label columns (pure relayout), final tiny log/mean stitch.
"""

import math
from contextlib import ExitStack

import numpy as np

B, T, C, L = 2048, 256, 37, 16
BLANK = 36
S = 2 * L + 1               # 33 layers
NCORES = 8
BC = B // NCORES            # 256 samples per core
G = 2                       # sample groups of 128 partitions per core
P = 128
NSCAN = 2 * T + 1           # 513: g0 t0..255, pad, g1 t0..255
CW = NSCAN + 1              # 514 stored columns per feed row
GBIAS = -1.0 * math.log(2.0)

_cache = {}


def _build():
    import concourse.bacc as bacc
    import concourse.mybir as mybir
    import concourse.tile as tile

    f32 = mybir.dt.float32
    bf16 = mybir.dt.bfloat16
    AF = mybir.ActivationFunctionType
    ALU = mybir.AluOpType
    AX = mybir.AxisListType

    nc = bacc.Bacc("TRN2", target_bir_lowering=False, debug=False)

    _gb = nc.alloc_sbuf_tensor("const-float32-gbias", [128, 1], f32)
    nc.gpsimd.memset(_gb.ap(), GBIAS)
    nc.const_aps.aps[(f32, GBIAS)] = _gb.ap()
    nc.all_engine_barrier(sem_only=True)

    # feed rows: row 0 = blank column, row 1+k = label-k column
    e17 = nc.dram_tensor("e17", [P, 17, CW], bf16, kind="ExternalInput")
    # class-major raw logits: [p, c, g, t]
    lgc = nc.dram_tensor("lgc", [P, C, G * T], bf16, kind="ExternalInput")
    msk = nc.dram_tensor("msk", [P, G * L], f32, kind="ExternalInput")
    idd = nc.dram_tensor("idd", [P, P], bf16, kind="ExternalInput")
    outv = nc.dram_tensor("outv", [P, G * 3], f32, kind="ExternalOutput")

    ECHUNKS = [(0, 1), (1, 2), (2, 9), (9, 17)]
    CCHUNKS = [(0, 10), (10, 19), (19, 28), (28, 37)]

    with tile.TileContext(nc) as tc, ExitStack() as ctx:
        pool1 = ctx.enter_context(tc.tile_pool(name="res", bufs=1))
        psum = ctx.enter_context(tc.tile_pool(name="ps", space="PSUM", bufs=1))

        e17t = pool1.tile([P, 17 * CW], bf16, tag="e17t")
        lgt = pool1.tile([P, C * G * T], bf16, tag="lgt")
        mtile = pool1.tile([P, G * L], f32, tag="mtile")
        ident = pool1.tile([P, P], bf16, tag="ident")
        z0 = pool1.tile([P, NSCAN], f32, tag="z0")
        l0 = pool1.tile([P, CW], bf16, tag="l0")
        lbufs = [pool1.tile([P, CW], bf16, tag=f"lb{i}", name=f"lb{i}")
                 for i in range(4)]
        vt = pool1.tile([P, NSCAN], bf16, tag="vt")
        lg37 = pool1.tile([P, G * T], f32, tag="lg37")
        lsum2 = pool1.tile([P, G], f32, tag="lsum2")
        outt = pool1.tile([P, G * 3], f32, tag="outt")

        ps512 = psum.tile([P, G * T], f32, tag="ps512")

        e17v = e17t[:].rearrange("p (r w) -> p r w", w=CW)
        lgv = lgt[:].rearrange("p (c n) -> p c n", n=G * T)

        # --- init constants (gpsimd to keep DVE free) ---
        warm = pool1.tile([P, 1], f32, tag="warm")
        nc.gpsimd.memset(warm[:], 0.0)
        nc.scalar.activation(warm[:], warm[:], AF.Exp, bias=GBIAS)
        nc.gpsimd.memset(z0[:], 0.0)
        nc.gpsimd.memset(z0[:, T + 1:T + 2], 1.0)
        for lb in lbufs:
            nc.gpsimd.memset(lb[:, 0:1], 0.0)
        nc.gpsimd.memset(l0[:, 0:1], 1.0)
        # --- phase 1: scan feed, head chunk first, exp in place ---
        for (r0, r1) in ECHUNKS:
            dst = e17v[:, r0:r1, :]
            nc.sync.dma_start(dst, e17.ap()[:, r0:r1, :])
            nc.scalar.activation(dst, dst, AF.Exp, bias=GBIAS)
        nc.sync.dma_start(mtile[:], msk.ap())
        nc.sync.dma_start(ident[:], idd.ap())

        # --- phase 2: denominator stream: DMA chunk -> exp -> PE matmuls ---
        for (c0, c1) in CCHUNKS:
            dst = lgv[:, c0:c1, :]
            nc.sync.dma_start(dst, lgc.ap()[:, c0:c1, :])
            nc.scalar.activation(dst, dst, AF.Exp, bias=GBIAS)
            for c in range(c0, c1):
                nc.tensor.matmul(
                    ps512[:], ident[:], lgv[:, c, :],
                    start=(c == 0), stop=(c == C - 1))

        # --- phase 3: the 33 layer scans on DVE ---
        def lbuf(s):
            return l0 if s == 0 else lbufs[(s - 1) % 4]

        def d1row(s):
            r = 0 if s % 2 == 0 else 1 + (s - 1) // 2
            return e17v[:, r, 0:NSCAN]

        for s in range(S):
            dst = lbuf(s)
            if s == 0:
                nc.vector.tensor_tensor_scan(
                    dst[:, 1:CW], z0[:], d1row(s), 1.0, ALU.add, ALU.mult)
                nc.vector.memset(dst[:, T + 1:T + 2], 1.0)
                continue
            if s == 1 or s % 2 == 0:
                d0 = lbuf(s - 1)[:, 0:NSCAN]
            else:
                k = (s - 1) // 2
                nc.vector.scalar_tensor_tensor(
                    vt[:, 0:T + 1], lbuf(s - 2)[:, 0:T + 1],
                    mtile[:, k:k + 1], lbuf(s - 1)[:, 0:T + 1],
                    ALU.mult, ALU.add)
                nc.vector.scalar_tensor_tensor(
                    vt[:, T + 1:NSCAN], lbuf(s - 2)[:, T + 1:NSCAN],
                    mtile[:, L + k:L + k + 1], lbuf(s - 1)[:, T + 1:NSCAN],
                    ALU.mult, ALU.add)
                d0 = vt[:, 0:NSCAN]
            nc.vector.tensor_tensor_scan(
                dst[:, 1:CW], d0, d1row(s), 0.0, ALU.add, ALU.mult)

        # --- phase 4: Ln(PSUM) with per-group accumulate (no DVE reduce) ---
        for g in range(G):
            nc.scalar.activation(
                lg37[:, g * T:(g + 1) * T], ps512[:, g * T:(g + 1) * T],
                AF.Ln, bias=0.0, accum_out=lsum2[:, g:g + 1])

        ot = outt[:].rearrange("p (g i) -> p g i", g=G)
        ov = outv.ap().rearrange("p (g i) -> p g i", g=G)
        l31, l32 = lbuf(S - 2), lbuf(S - 1)
        t31 = l31[:].rearrange("p (g t) -> p g t", g=G)[:, :, T]
        t32 = l32[:].rearrange("p (g t) -> p g t", g=G)[:, :, T]
        nc.vector.tensor_copy(ot[:, :, 2], lsum2[:])
        nc.sync.dma_start(ov[:, :, 2], ot[:, :, 2])
        nc.vector.tensor_copy(ot[:, :, 0], t31)
        nc.sync.dma_start(ov[:, :, 0], ot[:, :, 0])
        nc.vector.tensor_copy(ot[:, :, 1], t32)
        nc.sync.dma_start(ov[:, :, 1], ot[:, :, 1])

    nc.compile()
    return nc


def _host_prep(logits, targets):
    import ml_dtypes
    bf = ml_dtypes.bfloat16

    tgt = targets.reshape(B, L)
    mask = np.zeros((B, L), np.float32)
    mask[:, 1:] = (tgt[:, 1:] != tgt[:, :-1]).astype(np.float32)

    gath = np.take_along_axis(
        logits, np.broadcast_to(tgt[:, None, :], (B, T, L)), axis=2)  # [B,T,16]
    blank = logits[:, :, BLANK]                                       # [B,T]

    # feed [NCORES, 128, 17, 514]: row 0 blank, row 1+k label k;
    # cols 0..255 g0 t, col 256 pad(-100), 257..512 g1 t, col 513 pad
    feed = np.full((NCORES, P, 17, CW), -100.0, np.float32)
    bl = blank.reshape(NCORES, G, P, T)
    ga = gath.reshape(NCORES, G, P, T, L)
    feed[:, :, 0, 0:T] = bl[:, 0]
    feed[:, :, 0, T + 1:NSCAN] = bl[:, 1]
    feed[:, :, 1:, 0:T] = np.moveaxis(ga[:, 0], 3, 2)
    feed[:, :, 1:, T + 1:NSCAN] = np.moveaxis(ga[:, 1], 3, 2)

    # class-major logits [NCORES, 128, 37, 2*256]
    lg = np.ascontiguousarray(
        logits.reshape(NCORES, G, P, T, C).transpose(0, 2, 4, 1, 3)
    ).reshape(NCORES, P, C, G * T)

    mk = np.ascontiguousarray(
        mask.reshape(NCORES, G, P, L).transpose(0, 2, 1, 3)
    ).reshape(NCORES, P, G * L)

    return (np.ascontiguousarray(feed).astype(bf),
            lg.astype(bf), mk, np.eye(P, dtype=np.float32).astype(bf))


def kernel(logits, targets, input_lengths, target_lengths):
    logits = np.asarray(logits, np.float32)
    targets = np.asarray(targets, np.int32)
    assert logits.shape == (B, T, C)

    from concourse import bass_utils

    if "nc" not in _cache:
        _cache["nc"] = _build()
    nc = _cache["nc"]

    feed, lg, mk, ident = _host_prep(logits, targets)
    in_maps = []
    for ci in range(NCORES):
        in_maps.append({
            "e17": feed[ci],
            "lgc": lg[ci],
            "msk": mk[ci],
            "idd": ident,
        })
    res = bass_utils.run_bass_kernel_spmd(nc, in_maps, core_ids=list(range(NCORES)))
    outs = np.stack([r["outv"] for r in res.results])  # [NCORES, 128, 6]
    ov = outs.reshape(NCORES, P, G, 3).transpose(0, 2, 1, 3).reshape(B, 3)
    a31 = ov[:, 0].astype(np.float64)
    a32 = ov[:, 1].astype(np.float64)
    lz = ov[:, 2].astype(np.float64)
    ll = np.log(a31 + a32) - lz
    loss = np.mean(-ll / L)
    return np.float32(loss)


# revision 9
# speedup vs baseline: 1.0137x; 1.0137x over previous
"""CTC loss kernel for Trainium2 (8 NeuronCores, data-parallel over batch).

Algorithm: the CTC forward DP alpha[t, s] runs as 33 "layer scans" on DVE:
for each extended-target position s, the time recursion
    A[t, s] = (A[t-1, s-1] + m[s] * A[t-1, s-2] + A[t-1, s]) * E[t, s]
is a first-order affine recurrence in t evaluated with one stock DVE
`tensor_tensor_scan` per layer (state = (d0 + state) * d1), both sample
groups concatenated into a single 513-column scan.

v2 changes vs the first working version:
  - feeds shipped in bf16 (half the HBM traffic),
  - the scan feed is shipped transposed ([label-row, time]) so every
    layer's d1 operand is contiguous,
  - the 37-class softmax-denominator reduction runs on the Tensor engine
    as 37 identity-stationary matmuls accumulating into one PSUM bank
    (frees ~21us of DVE time),
  - Ln reads the PSUM bank directly on ACT,
  - scans start after a small 3-row head chunk of the feed is loaded.

Scaled linear domain: E = exp(logit - ln2); the 2^-1 scaling cancels in
ll = log(A_final) - sum_t log(sum37).  Host side: shard batch 2048 ->
8 cores x 256 samples (2 groups of 128 partitions), pre-gather the 17
per-sample label columns (pure relayout), final tiny log/mean stitch.
"""

import math
from contextlib import ExitStack

import numpy as np

B, T, C, L = 2048, 256, 37, 16
BLANK = 36
S = 2 * L + 1               # 33 layers
NCORES = 8
BC = B // NCORES            # 256 samples per core
G = 2                       # sample groups of 128 partitions per core
P = 128
NSCAN = 2 * T + 1           # 513: g0 t0..255, pad, g1 t0..255
CW = NSCAN + 1              # 514 stored columns per feed row
GBIAS = -1.0 * math.log(2.0)

_cache = {}


def _build():
    import concourse.bacc as bacc
    import concourse.mybir as mybir
    import concourse.tile as tile

    f32 = mybir.dt.float32
    bf16 = mybir.dt.bfloat16
    AF = mybir.ActivationFunctionType
    ALU = mybir.AluOpType
    AX = mybir.AxisListType

    nc = bacc.Bacc("TRN2", target_bir_lowering=False, debug=False)

    _gb = nc.alloc_sbuf_tensor("const-float32-gbias", [128, 1], f32)
    nc.vector.memset(_gb.ap(), GBIAS)
    nc.const_aps.aps[(f32, GBIAS)] = _gb.ap()

    # feed rows: row 0 = blank column, row 1+k = label-k column
    e17 = nc.dram_tensor("e17", [P, 17, CW], bf16, kind="ExternalInput")
    # class-major raw logits: [p, c, g, t]
    lgc = nc.dram_tensor("lgc", [P, C, G * T], bf16, kind="ExternalInput")
    msk = nc.dram_tensor("msk", [P, G * L], f32, kind="ExternalInput")
    idd = nc.dram_tensor("idd", [P, P], bf16, kind="ExternalInput")
    outv = nc.dram_tensor("outv", [P, G * 3], f32, kind="ExternalOutput")

    ECHUNKS = [(0, 1), (1, 2), (2, 9), (9, 17)]
    CCHUNKS = [(0, 10), (10, 19), (19, 28), (28, 37)]

    with tile.TileContext(nc) as tc, ExitStack() as ctx:
        pool1 = ctx.enter_context(tc.tile_pool(name="res", bufs=1))
        psum = ctx.enter_context(tc.tile_pool(name="ps", space="PSUM", bufs=1))

        e17t = pool1.tile([P, 17 * CW], bf16, tag="e17t")
        lgt = pool1.tile([P, C * G * T], bf16, tag="lgt")
        mtile = pool1.tile([P, G * L], f32, tag="mtile")
        ident = pool1.tile([P, P], bf16, tag="ident")
        z0 = pool1.tile([P, NSCAN], f32, tag="z0")
        l0 = pool1.tile([P, CW], bf16, tag="l0")
        lbufs = [pool1.tile([P, CW], bf16, tag=f"lb{i}", name=f"lb{i}")
                 for i in range(4)]
        vt = pool1.tile([P, NSCAN], bf16, tag="vt")
        lg37 = pool1.tile([P, G * T], f32, tag="lg37")
        lsum2 = pool1.tile([P, G], f32, tag="lsum2")
        outt = pool1.tile([P, G * 3], f32, tag="outt")

        ps512 = psum.tile([P, G * T], f32, tag="ps512")

        e17v = e17t[:].rearrange("p (r w) -> p r w", w=CW)
        lgv = lgt[:].rearrange("p (c n) -> p c n", n=G * T)

        # --- init constants (gpsimd to keep DVE free) ---
        warm = pool1.tile([P, 1], f32, tag="warm")
        nc.vector.memset(warm[:], 0.0)
        nc.scalar.activation(warm[:], warm[:], AF.Exp, bias=GBIAS)
        nc.gpsimd.memset(z0[:], 0.0)
        nc.gpsimd.memset(z0[:, T + 1:T + 2], 1.0)
        for lb in lbufs:
            nc.gpsimd.memset(lb[:, 0:1], 0.0)
        # --- phase 1: scan feed, head chunk first, exp in place ---
        for (r0, r1) in ECHUNKS:
            dst = e17v[:, r0:r1, :]
            nc.sync.dma_start(dst, e17.ap()[:, r0:r1, :])
            nc.scalar.activation(dst, dst, AF.Exp, bias=GBIAS)
        nc.sync.dma_start(mtile[:], msk.ap())
        nc.sync.dma_start(ident[:], idd.ap())

        # --- phase 2: denominator stream: DMA chunk -> exp -> PE matmuls ---
        for (c0, c1) in CCHUNKS:
            dst = lgv[:, c0:c1, :]
            nc.sync.dma_start(dst, lgc.ap()[:, c0:c1, :])
            nc.scalar.activation(dst, dst, AF.Exp, bias=GBIAS)
            for c in range(c0, c1):
                nc.tensor.matmul(
                    ps512[:], ident[:], lgv[:, c, :],
                    start=(c == 0), stop=(c == C - 1))

        # --- phase 3: the 33 layer scans on DVE ---
        def lbuf(s):
            return l0 if s == 0 else lbufs[(s - 1) % 4]

        def d1row(s):
            r = 0 if s % 2 == 0 else 1 + (s - 1) // 2
            return e17v[:, r, 0:NSCAN]

        for s in range(S):
            dst = lbuf(s)
            if s == 0:
                nc.vector.tensor_tensor_scan(
                    dst[:, 1:CW], z0[:], d1row(s), 1.0, ALU.add, ALU.mult)
                nc.vector.memset(dst[:, 0:1], 1.0)
                nc.vector.memset(dst[:, T + 1:T + 2], 1.0)
                continue
            if s == 1 or s % 2 == 0:
                d0 = lbuf(s - 1)[:, 0:NSCAN]
            else:
                k = (s - 1) // 2
                nc.vector.scalar_tensor_tensor(
                    vt[:, 0:T + 1], lbuf(s - 2)[:, 0:T + 1],
                    mtile[:, k:k + 1], lbuf(s - 1)[:, 0:T + 1],
                    ALU.mult, ALU.add)
                nc.vector.scalar_tensor_tensor(
                    vt[:, T + 1:NSCAN], lbuf(s - 2)[:, T + 1:NSCAN],
                    mtile[:, L + k:L + k + 1], lbuf(s - 1)[:, T + 1:NSCAN],
                    ALU.mult, ALU.add)
                d0 = vt[:, 0:NSCAN]
            nc.vector.tensor_tensor_scan(
                dst[:, 1:CW], d0, d1row(s), 0.0, ALU.add, ALU.mult)

        # --- phase 4: Ln(PSUM) with per-group accumulate (no DVE reduce) ---
        for g in range(G):
            nc.scalar.activation(
                lg37[:, g * T:(g + 1) * T], ps512[:, g * T:(g + 1) * T],
                AF.Ln, bias=0.0, accum_out=lsum2[:, g:g + 1])

        ot = outt[:].rearrange("p (g i) -> p g i", g=G)
        l31, l32 = lbuf(S - 2), lbuf(S - 1)
        t31 = l31[:].rearrange("p (g t) -> p g t", g=G)[:, :, T]
        t32 = l32[:].rearrange("p (g t) -> p g t", g=G)[:, :, T]
        nc.vector.tensor_copy(ot[:, :, 0], t31)
        nc.vector.tensor_copy(ot[:, :, 1], t32)
        nc.vector.tensor_copy(ot[:, :, 2], lsum2[:])
        nc.sync.dma_start(
            outv.ap().rearrange("p (g i) -> p g i", g=G), ot)

    nc.compile()
    return nc


def _host_prep(logits, targets):
    import ml_dtypes
    bf = ml_dtypes.bfloat16

    tgt = targets.reshape(B, L)
    mask = np.zeros((B, L), np.float32)
    mask[:, 1:] = (tgt[:, 1:] != tgt[:, :-1]).astype(np.float32)

    gath = np.take_along_axis(
        logits, np.broadcast_to(tgt[:, None, :], (B, T, L)), axis=2)  # [B,T,16]
    blank = logits[:, :, BLANK]                                       # [B,T]

    # feed [NCORES, 128, 17, 514]: row 0 blank, row 1+k label k;
    # cols 0..255 g0 t, col 256 pad(-100), 257..512 g1 t, col 513 pad
    feed = np.full((NCORES, P, 17, CW), -100.0, np.float32)
    bl = blank.reshape(NCORES, G, P, T)
    ga = gath.reshape(NCORES, G, P, T, L)
    feed[:, :, 0, 0:T] = bl[:, 0]
    feed[:, :, 0, T + 1:NSCAN] = bl[:, 1]
    feed[:, :, 1:, 0:T] = np.moveaxis(ga[:, 0], 3, 2)
    feed[:, :, 1:, T + 1:NSCAN] = np.moveaxis(ga[:, 1], 3, 2)

    # class-major logits [NCORES, 128, 37, 2*256]
    lg = np.ascontiguousarray(
        logits.reshape(NCORES, G, P, T, C).transpose(0, 2, 4, 1, 3)
    ).reshape(NCORES, P, C, G * T)

    mk = np.ascontiguousarray(
        mask.reshape(NCORES, G, P, L).transpose(0, 2, 1, 3)
    ).reshape(NCORES, P, G * L)

    return (np.ascontiguousarray(feed).astype(bf),
            lg.astype(bf), mk, np.eye(P, dtype=np.float32).astype(bf))


def kernel(logits, targets, input_lengths, target_lengths):
    logits = np.asarray(logits, np.float32)
    targets = np.asarray(targets, np.int32)
    assert logits.shape == (B, T, C)

    from concourse import bass_utils

    if "nc" not in _cache:
        _cache["nc"] = _build()
    nc = _cache["nc"]

    feed, lg, mk, ident = _host_prep(logits, targets)
    in_maps = []
    for ci in range(NCORES):
        in_maps.append({
            "e17": feed[ci],
            "lgc": lg[ci],
            "msk": mk[ci],
            "idd": ident,
        })
    res = bass_utils.run_bass_kernel_spmd(nc, in_maps, core_ids=list(range(NCORES)))
    outs = np.stack([r["outv"] for r in res.results])  # [NCORES, 128, 6]
    ov = outs.reshape(NCORES, P, G, 3).transpose(0, 2, 1, 3).reshape(B, 3)
    a31 = ov[:, 0].astype(np.float64)
    a32 = ov[:, 1].astype(np.float64)
    lz = ov[:, 2].astype(np.float64)
    ll = np.log(a31 + a32) - lz
    loss = np.mean(-ll / L)
    return np.float32(loss)
